# revision 1
# baseline (speedup 1.0000x reference)
"""Trainium2 Bass kernel for nn_MemoryReader (sparse_attention).

Reference computation (per batch b):
  s[m,q]  = sum_c K_M[b,c,m] * K_Q[b,c,q] / sqrt(64)        m in [0,9216), q in [0,2304)
  attn    = softmax over m
  mem[c,q]= sum_m V_M[b,c,m] * attn[m,q]                    c in [0,128)
  E       = concat([mem, V_Q[b]], ch)                       [256, q]
  out     = relu(bn_scale * (conv_w @ E) + bn_shift)        [64, q]

Sharding: 8 cores = (B=4) x (Q halves of 1152). Fully data-parallel, no
collectives. Within a core, Q is processed in 2 passes of 576 columns
(PSUM capacity), softmax over the full M=9216 without max-subtraction
(scores are ~N(0,1): exp is safe in fp32/bf16 range).

Per-core inputs (host-prepared), packed to minimize DMA semaphore domains
(walrus allows a single sync-wait per compute instruction and a limited
wait list on the kernel-tail drain):
  kmq [128, 5824] bf16 : cols 0:4608   km_packed (even m-tiles on
                         partitions 0-63, odd on 64-127 -> QK matmul pairs
                         run concurrently in PE row groups, contract=64)
                         cols 4608:5760 kq_dup (K_Q slice, pre-scaled by
                         1/sqrt(64), duplicated on partitions 64-127)
                         cols 5760:5824 w1t (BN-folded conv W for the mem
                         half, transposed)
  vt  [128, 9216] bf16 : V_M transposed per 128-tile: vt[p, t*128+c] =
                         V_M[c, t*128+p]  (PV lhsT, contract on m)
  vqw [128, 1217] f32  : cols 0:1152 V_Q slice, cols 1152:1216 w2t,
                         col 1216 (rows 0:64) BN shift
Output: out [64, 1152] f32.
"""

import numpy as np
import ml_dtypes

import concourse.bass as bass
from concourse import bacc
import concourse.mybir as mybir
import concourse.tile as tile
from concourse.tile_rust import add_dep_helper

B, C_K, C_V, NN, H, W = 4, 64, 128, 4, 48, 48
M = NN * H * W          # 9216
Q = H * W               # 2304
QH = Q // 2             # 1152 per core
QP = QH // 2            # 576 per in-kernel pass
OUT_CH = 64
BN_EPS = 1e-5
NCORES = 8
MT = M // 128           # 72 m-tiles
KMW = (MT // 2) * 128   # 4608
BF16 = mybir.dt.bfloat16
F32 = mybir.dt.float32
AF = mybir.ActivationFunctionType


def _emit(nc, aps, reps=1):
    kmq, vt, vqw, out = aps
    with tile.TileContext(nc) as tc:
        with (
            tc.tile_pool(name="consts", bufs=1) as consts,
            tc.tile_pool(name="pp", bufs=8) as pp,
            tc.tile_pool(name="epi", bufs=2) as epi,
            tc.tile_pool(name="obuf", bufs=1) as obuf,
            tc.tile_pool(name="dacc", bufs=2) as daccp,
            tc.tile_pool(name="spool", bufs=3, space="PSUM") as spool,
            tc.tile_pool(name="mpool", bufs=1, space="PSUM") as mpool,
        ):
            kmq_t = consts.tile([128, KMW + QH + OUT_CH], BF16)
            vt_t = consts.tile([128, M], BF16)
            vqw_t = consts.tile([128, QH + OUT_CH + 1], F32)
            ones_t = consts.tile([128, OUT_CH], BF16)

            # chunked loads: early m-tiles unblock before the tails arrive
            nc.sync.dma_start(out=kmq_t[:, KMW:KMW + QP], in_=kmq[:, KMW:KMW + QP])
            nc.sync.dma_start(out=kmq_t[:, 0:1152], in_=kmq[:, 0:1152])
            nc.sync.dma_start(out=kmq_t[:, KMW + QP:], in_=kmq[:, KMW + QP:])
            nc.sync.dma_start(out=vt_t[:, 0:2304], in_=vt[:, 0:2304])
            nc.sync.dma_start(out=vqw_t, in_=vqw)
            nc.sync.dma_start(out=kmq_t[:, 1152:KMW], in_=kmq[:, 1152:KMW])
            for i in range(1, 4):
                nc.sync.dma_start(out=vt_t[:, i * 2304:(i + 1) * 2304],
                                  in_=vt[:, i * 2304:(i + 1) * 2304])
            # DVE rewrites vqw and then produces ones: a single DVE
            # semaphore covers both; PE observes it via one dummy ldweights
            vqw_cp = nc.vector.tensor_copy(out=vqw_t, in_=vqw_t)
            ones_ms = nc.vector.memset(ones_t, 1.0)
            # order the memset after the copy on the Pool queue so one
            # ldweights absorber (Pool >= memset) covers the vqw rewrite too
            add_dep_helper(ones_ms.ins, vqw_cp.ins, sync=False,
                           reason="ones memset after vqw copy")

            kq0 = KMW                     # kq columns inside kmq_t
            w1c = KMW + QH                # w1t columns inside kmq_t
            vq_t = vqw_t[:, 0:QH]
            w2t_t = vqw_t[:, QH:QH + OUT_CH]
            shift_t = vqw_t[0:OUT_CH, QH + OUT_CH:QH + OUT_CH + 1]

            o_t = obuf.tile([OUT_CH, QH], F32, tag="o")

            LAG = 4
            p_hist = []
            for rep in range(reps):
              for p in range(2):
                  qs = p * QP
                  # [128, 1024] = 2 PSUM banks, one accumulation group per
                  # bank: mem in cols 0:512 (bank 0) + 512:576 (bank 1).
                  # The softmax denominator accumulates on DVE (pair-tree).
                  mem_t = mpool.tile([128, 1024], F32, tag="mem")
                  g_t = daccp.tile([128, QP], BF16, tag="g")
                  p_by_mt = {}
                  # software pipeline: emit QK(mt)/exp(mt) LAG steps ahead of
                  # PV(mt-LAG) so the in-order PE queue never stalls on exp
                  for mt in range(MT + LAG):
                    if mt < MT:
                      bp = 64 * (mt % 2)
                      cl = (mt // 2) * 128
                      lhs = kmq_t[bp:bp + 64, cl:cl + 128]
                      if len(p_hist) >= 3:
                          # absorbs the s-slot WAR (the exp lagging exactly a
                          # full spool rotation) into a 1-wait ldweights; the
                          # QK below then only carries its PSUM WAW wait
                          nc.tensor.ldweights(p_hist[-3][0:1, 0:2])
                      s_t = spool.tile([128, 1024], F32, tag="s")
                      qk0 = nc.tensor.matmul(
                          s_t[:, 0:512], lhs,
                          kmq_t[bp:bp + 64, kq0 + qs:kq0 + qs + 512],
                          start=True, stop=True)
                      qk1 = nc.tensor.matmul(
                          s_t[:, 512:576], lhs,
                          kmq_t[bp:bp + 64, kq0 + qs + 512:kq0 + qs + QP],
                          start=True, stop=True)
                      if p == 0 and mt == 0 and rep == 0:
                          # absorbers: PE observes the vt DMA and the DVE
                          # (vqw copy + ones memset) with one wait each
                          nc.tensor.ldweights(vt_t[:, 0:1])
                          nc.tensor.ldweights(ones_t[:, 0:1])
                      p_t = pp.tile([128, QP], BF16, tag="p")
                      nc.scalar.activation(out=p_t, in_=s_t[:, 0:QP], func=AF.Exp)
                      p_by_mt[mt] = p_t
                      p_hist.append(p_t)
                      if len(p_hist) > 6:
                          p_hist.pop(0)
                      # d pair-tree on DVE: e = p(even)+p(odd) waits only on
                      # ACT; the g fold waits only on DVE
                      if mt % 2 == 1:
                          if mt == 1:
                              nc.vector.tensor_add(g_t, p_hist[-2], p_t)
                          else:
                              e_t = pp.tile([128, QP], BF16, tag="e")
                              nc.vector.tensor_add(e_t, p_hist[-2], p_t)
                              nc.vector.tensor_add(g_t, g_t, e_t)
                    if mt >= LAG:
                      pv = mt - LAG
                      pvp = p_by_mt.pop(pv)
                      vl = vt_t[:, pv * 128:(pv + 1) * 128]
                      st, sp = (pv == 0), (pv == MT - 1)
                      nc.tensor.matmul(
                          mem_t[:, 0:512], vl, pvp[:, 0:512], start=st, stop=sp)
                      nc.tensor.matmul(
                          mem_t[:, 512:QP], vl, pvp[:, 512:QP], start=st, stop=sp)

                  # epilogue for this pass; the y1/y2/d_red matmuls reuse the
                  # now-free spool / mem_t PSUM banks
                  mem_sb = epi.tile([128, QP], BF16, tag="mem_sb")
                  nc.vector.tensor_copy(out=mem_sb, in_=mem_t[:, 0:QP])
                  # d_red and y1 reuse the two freed mem_t banks (d on
                  # partitions 64:128, y1 on 0:64) so the spool slots stay
                  # available for the next pass's QKs
                  nc.tensor.matmul(mem_t[64:128, 0:512], ones_t[:, 0:64],
                                   g_t[:, 0:512], start=True, stop=True)
                  nc.tensor.matmul(mem_t[64:128, 512:QP], ones_t[:, 0:64],
                                   g_t[:, 512:QP], start=True, stop=True)
                  r_t = epi.tile([64, QP], F32, tag="r")
                  nc.vector.reciprocal(out=r_t, in_=mem_t[64:128, 0:QP])

                  y2 = spool.tile([64, 1024], F32, tag="s")
                  nc.tensor.matmul(y2[0:64, 0:512], w2t_t[:, 0:64],
                                   vq_t[:, qs:qs + 512], start=True, stop=True)
                  nc.tensor.matmul(y2[0:64, 512:QP], w2t_t[:, 0:64],
                                   vq_t[:, qs + 512:qs + QP], start=True, stop=True)
                  nc.tensor.ldweights(mem_sb[:, 0:1])
                  nc.tensor.matmul(mem_t[0:64, 0:512], kmq_t[:, w1c:w1c + OUT_CH],
                                   mem_sb[:, 0:512], start=True, stop=True)
                  nc.tensor.matmul(mem_t[0:64, 512:QP], kmq_t[:, w1c:w1c + OUT_CH],
                                   mem_sb[:, 512:QP], start=True, stop=True)

                  # DVE observes its own r_t value so the u-mul needs only the
                  # PE wait (y1, which also covers y2)
                  rabs = epi.tile([64, 2], F32, tag="rabs")
                  nc.vector.tensor_copy(out=rabs, in_=r_t[:, 0:2])
                  u_t = epi.tile([64, QP], F32, tag="u")
                  nc.vector.tensor_mul(u_t, mem_t[0:64, 0:QP], r_t)
                  v_t = epi.tile([64, QP], F32, tag="v")
                  nc.vector.tensor_add(v_t, u_t, y2[0:64, 0:QP])
                  nc.vector.tensor_scalar(
                      out=o_t[:, qs:qs + QP], in0=v_t, scalar1=shift_t,
                      scalar2=0.0, op0=mybir.AluOpType.add,
                      op1=mybir.AluOpType.max)
            nc.sync.dma_start(out=out, in_=o_t)


def _build_nc(reps=1):
    nc = bacc.Bacc("TRN2", target_bir_lowering=False, debug=False)
    kmq = nc.dram_tensor("kmq", [128, KMW + QH + OUT_CH], BF16,
                         kind="ExternalInput").ap()
    vt = nc.dram_tensor("vt", [128, M], BF16, kind="ExternalInput").ap()
    vqw = nc.dram_tensor("vqw", [128, QH + OUT_CH + 1], F32,
                         kind="ExternalInput").ap()
    out = nc.dram_tensor("out", [OUT_CH, QH], F32, kind="ExternalOutput").ap()
    _emit(nc, (kmq, vt, vqw, out), reps=reps)
    nc.compile()
    return nc


def prepare_in_maps(K_M, V_M, K_Q, V_Q, conv_w, bn_gamma, bn_beta, bn_mean, bn_var):
    """Host-side shard + layout prep. Returns list of 8 per-core input dicts."""
    bf16 = ml_dtypes.bfloat16
    K_M = np.asarray(K_M, np.float32)
    V_M = np.asarray(V_M, np.float32)
    K_Q = np.asarray(K_Q, np.float32)
    V_Q = np.asarray(V_Q, np.float32)
    conv_w = np.asarray(conv_w, np.float32)
    scale = np.asarray(bn_gamma, np.float32) / np.sqrt(
        np.asarray(bn_var, np.float32) + BN_EPS)
    shift = (np.asarray(bn_beta, np.float32)
             - np.asarray(bn_mean, np.float32) * scale)
    w_eff = conv_w * scale[:, None]
    w1t = np.ascontiguousarray(w_eff[:, :C_V].T)          # [128, 64]
    w2t = np.ascontiguousarray(w_eff[:, C_V:].T)          # [128, 64]

    in_maps = []
    for b in range(B):
        km_full = K_M[b].reshape(C_K, M)                  # [64, 9216]
        km_r = km_full.reshape(C_K, MT, 128)
        km_packed = np.empty((128, KMW), np.float32)
        km_packed[0:64] = km_r[:, 0::2, :].reshape(C_K, -1)
        km_packed[64:128] = km_r[:, 1::2, :].reshape(C_K, -1)

        v_full = V_M[b].reshape(C_V, M)
        vt = np.ascontiguousarray(
            v_full.reshape(C_V, MT, 128).transpose(2, 1, 0).reshape(128, M)
        ).astype(bf16)

        kq_full = K_Q[b].reshape(C_K, Q) * (1.0 / np.sqrt(C_K))
        vq_full = V_Q[b].reshape(C_V, Q)
        for h in range(2):
            sl = slice(h * QH, (h + 1) * QH)
            kq_half = kq_full[:, sl]
            kmq = np.empty((128, KMW + QH + OUT_CH), np.float32)
            kmq[:, 0:KMW] = km_packed
            kmq[0:64, KMW:KMW + QH] = kq_half
            kmq[64:128, KMW:KMW + QH] = kq_half
            kmq[:, KMW + QH:] = w1t
            vqw = np.zeros((128, QH + OUT_CH + 1), np.float32)
            vqw[:, 0:QH] = vq_full[:, sl]
            vqw[:, QH:QH + OUT_CH] = w2t
            vqw[0:OUT_CH, QH + OUT_CH] = shift
            in_maps.append({
                "kmq": kmq.astype(bf16),
                "vt": vt,
                "vqw": vqw,
            })
    return in_maps


def assemble_output(results):
    """results: list of 8 dicts with 'out' [64, 1152] -> [4, 64, 48, 48] f32."""
    out = np.empty((B, OUT_CH, Q), np.float32)
    for c in range(NCORES):
        b, h = c // 2, c % 2
        out[b, :, h * QH:(h + 1) * QH] = results[c]["out"]
    return out.reshape(B, OUT_CH, H, W)


_RUNNERS = {}


def _get_runner(reps=1):
    """Build the Bass module + a cached sharded jit callable (compile once)."""
    if reps in _RUNNERS:
        return _RUNNERS[reps]
    import jax
    from jax.sharding import Mesh, PartitionSpec
    from jax.experimental.shard_map import shard_map
    from concourse import bass2jax

    nc = _build_nc(reps=reps)
    bass2jax.install_neuronx_cc_hook()

    partition_name = nc.partition_id_tensor.name if nc.partition_id_tensor else None
    in_names, out_names, out_avals, zero_outs = [], [], [], []
    for alloc in nc.m.functions[0].allocations:
        if not isinstance(alloc, mybir.MemoryLocationSet):
            continue
        name = alloc.memorylocations[0].name
        if alloc.kind == "ExternalInput":
            if name != partition_name:
                in_names.append(name)
        elif alloc.kind == "ExternalOutput":
            out_names.append(name)
            shape = tuple(alloc.tensor_shape)
            dtype = mybir.dt.np(alloc.dtype)
            out_avals.append(jax.core.ShapedArray(shape, dtype))
            zero_outs.append(np.zeros(shape, dtype))
    n_params = len(in_names)
    n_outs = len(out_avals)
    all_in_names = in_names + out_names
    if partition_name is not None:
        all_in_names = all_in_names + [partition_name]

    def _body(*args):
        operands = list(args)
        if partition_name is not None:
            operands.append(bass2jax.partition_id_tensor())
        outs = bass2jax._bass_exec_p.bind(
            *operands,
            out_avals=tuple(out_avals),
            in_names=tuple(all_in_names),
            out_names=tuple(out_names),
            lowering_input_output_aliases=(),
            sim_require_finite=True,
            sim_require_nnan=True,
            nc=nc,
        )
        return tuple(outs)

    devices = jax.devices()[:NCORES]
    assert len(devices) == NCORES, f"need {NCORES} devices, got {len(jax.devices())}"
    mesh = Mesh(np.asarray(devices), ("core",))
    in_specs = (PartitionSpec("core"),) * (n_params + n_outs)
    out_specs = (PartitionSpec("core"),) * n_outs
    donate = tuple(range(n_params, n_params + n_outs))
    sharded = jax.jit(
        shard_map(_body, mesh=mesh, in_specs=in_specs, out_specs=out_specs,
                  check_rep=False),
        donate_argnums=donate, keep_unused=True,
    )
    _RUNNERS[reps] = (sharded, in_names, out_names, out_avals, zero_outs)
    return _RUNNERS[reps]


def run_cores(in_maps):
    """Run the 8-core SPMD program; returns per-core output dicts."""
    sharded, in_names, out_names, out_avals, zero_outs = _get_runner()
    concat_in = [
        np.concatenate([np.asarray(in_maps[c][n]) for c in range(NCORES)], axis=0)
        for n in in_names
    ]
    concat_zeros = [
        np.zeros((NCORES * z.shape[0], *z.shape[1:]), z.dtype) for z in zero_outs
    ]
    out_arrs = sharded(*concat_in, *concat_zeros)
    return [
        {
            name: np.asarray(out_arrs[i]).reshape(NCORES, *out_avals[i].shape)[c]
            for i, name in enumerate(out_names)
        }
        for c in range(NCORES)
    ]


def kernel(K_M, V_M, K_Q, V_Q, conv_w, bn_gamma, bn_beta, bn_mean, bn_var):
    in_maps = prepare_in_maps(K_M, V_M, K_Q, V_Q, conv_w,
                              bn_gamma, bn_beta, bn_mean, bn_var)
    results = run_cores(in_maps)
    return assemble_output(results)



# revision 24
# speedup vs baseline: 1.8163x; 1.8163x over previous
"""Trainium2 Bass kernel for nn_MemoryReader (sparse_attention).

Reference computation (per batch b):
  s[m,q]  = sum_c K_M[b,c,m] * K_Q[b,c,q] / sqrt(64)        m in [0,9216), q in [0,2304)
  attn    = softmax over m
  mem[c,q]= sum_m V_M[b,c,m] * attn[m,q]                    c in [0,128)
  E       = concat([mem, V_Q[b]], ch)                       [256, q]
  out     = relu(bn_scale * (conv_w @ E) + bn_shift)        [64, q]

Sharding: 8 cores = (B=4) x (Q halves of 1152). Fully data-parallel, no
collectives. Within a core, Q is processed in 2 passes of 576 columns.

m-tiles are processed in PAIRS (even tile on partitions 0:64 of the packed
K, odd on 64:128 -> the two QK matmuls run concurrently in PE row groups).
Each pair's scores live in one 3-bank PSUM slot laid out
  [e512 | o512 | e64 o64 pad]  (cols 0:512 / 512:1024 / 1024:1152)
so ONE activation instruction computes exp for the whole pair (N=1152),
amortizing the ACT access overhead. Two slots rotate (6 banks) + the PV
accumulator (2 banks) fill all 8 PSUM banks.

The softmax denominator accumulates on DVE (g += p per pair, bf16 2x mode)
and is reduced over partitions by a ones-matmul at pass end. To balance
engine load, a subset of pairs computes exp on DVE instead of ACT via a
Schraudolph-style bit trick: int16 = rint(s*log2e*128 + (16256+C)) viewed
as bf16 is 2^(s*log2e) with ~2-4% element error; softmax cancels the
common mode and the attention branch is ~1% of the output magnitude, so
the end-to-end error stays ~2e-4 (validated vs the reference).

Per-core inputs (host-prepared):
  kmq [128, 5824] bf16 : cols 0:4608   km_packed (even m-tiles on
                         partitions 0-63, odd on 64-127, contract=64)
                         cols 4608:5760 kq_dup (K_Q slice, pre-scaled by
                         1/sqrt(64), duplicated on partitions 64-127)
                         cols 5760:5824 w1t (BN-folded conv W for the mem
                         half, transposed)
  vt  [128, 9216] bf16 : V_M transposed per 128-tile: vt[p, t*128+c] =
                         V_M[c, t*128+p]  (PV lhsT, contract on m)
  vqw [128, 1217] f32  : cols 0:1152 V_Q slice, cols 1152:1216 w2t,
                         col 1216 (rows 0:64) BN shift
Output: out [64, 1152] f32.
"""

import numpy as np
import ml_dtypes

import concourse.bass as bass
from concourse import bacc
import concourse.mybir as mybir
import concourse.tile as tile
from concourse.tile_rust import add_dep_helper

B, C_K, C_V, NN, H, W = 4, 64, 128, 4, 48, 48
M = NN * H * W          # 9216
Q = H * W               # 2304
QH = Q // 2             # 1152 per core
QP = QH // 2            # 576 per in-kernel pass
OUT_CH = 64
BN_EPS = 1e-5
NCORES = 8
MT = M // 128           # 72 m-tiles
NPAIR = MT // 2         # 36 m-tile pairs per pass
KMW = (MT // 2) * 128   # 4608
BF16 = mybir.dt.bfloat16
F32 = mybir.dt.float32
I16 = mybir.dt.int16
AF = mybir.ActivationFunctionType

# Schraudolph exp: int16 bits = s*log2e*128 + 16256 + C viewed as bf16
SCHRAUD_A = float(np.float32(np.log2(np.e) * 128.0))
SCHRAUD_B = float(np.float32(16256.0 - 7.5))
# exp split point: ACT exps cols [0:XSPLIT] natively, DVE does the rest
# via the Schraudolph bit trick (engine load balance)
XSPLIT = 1024


def _emit(nc, aps, reps=1):
    kmq, vt, vqw, out = aps
    with tile.TileContext(nc) as tc:
        with (
            tc.tile_pool(name="consts", bufs=1) as consts,
            tc.tile_pool(name="pp", bufs=6) as pp,
            tc.tile_pool(name="pq", bufs=6) as pq,
            tc.tile_pool(name="epi", bufs=2) as epi,
            tc.tile_pool(name="obuf", bufs=1) as obuf,
            tc.tile_pool(name="dacc", bufs=2) as daccp,
            tc.tile_pool(name="spool", bufs=2, space="PSUM") as spool,
            tc.tile_pool(name="mpool", bufs=1, space="PSUM") as mpool,
        ):
            kmq_t = consts.tile([128, KMW + QH + OUT_CH], BF16)
            vt_t = consts.tile([128, M], BF16)
            vqw_t = consts.tile([128, QH + OUT_CH + 1], F32)
            ones_t = consts.tile([128, OUT_CH], BF16)

            # chunked loads: early m-tiles unblock before the tails arrive
            nc.sync.dma_start(out=kmq_t[:, KMW:KMW + QP], in_=kmq[:, KMW:KMW + QP])
            nc.sync.dma_start(out=kmq_t[:, 0:1152], in_=kmq[:, 0:1152])
            nc.sync.dma_start(out=kmq_t[:, KMW + QP:], in_=kmq[:, KMW + QP:])
            nc.sync.dma_start(out=vqw_t, in_=vqw)
            nc.sync.dma_start(out=vt_t[:, 0:2304], in_=vt[:, 0:2304])
            nc.sync.dma_start(out=kmq_t[:, 1152:KMW], in_=kmq[:, 1152:KMW])
            for i in range(1, 4):
                nc.sync.dma_start(out=vt_t[:, i * 2304:(i + 1) * 2304],
                                  in_=vt[:, i * 2304:(i + 1) * 2304])
            # DVE rewrites vqw and then produces ones: a single DVE
            # semaphore covers both; PE observes it via one dummy ldweights
            vqw_cp = nc.vector.tensor_copy(out=vqw_t, in_=vqw_t)
            ones_ms = nc.vector.memset(ones_t, 1.0)
            add_dep_helper(ones_ms.ins, vqw_cp.ins, sync=False,
                           reason="ones memset after vqw copy")

            kq0 = KMW                     # kq columns inside kmq_t
            w1c = KMW + QH                # w1t columns inside kmq_t
            vq_t = vqw_t[:, 0:QH]
            w2t_t = vqw_t[:, QH:QH + OUT_CH]
            shift_t = vqw_t[0:OUT_CH, QH + OUT_CH:QH + OUT_CH + 1]

            o_t = obuf.tile([OUT_CH, QH], F32, tag="o")

            def emit_epilogue(qs, mem_t, g_t, gb_t, mem_sb, y2_sb):
                # d (softmax denominator) over g: partition-reduce via
                # ones-matmuls accumulating even+odd halves; lives in the
                # freed mem banks (partitions 64:128)
                nc.tensor.matmul(mem_t[64:128, 0:512], ones_t[:, 0:64],
                                 g_t[:, 0:512], start=True, stop=False)
                nc.tensor.matmul(mem_t[64:128, 0:512], ones_t[:, 0:64],
                                 g_t[:, 512:1024], start=False, stop=True)
                nc.tensor.matmul(mem_t[64:128, 512:576], ones_t[:, 0:64],
                                 gb_t[:, 0:64], start=True, stop=False)
                nc.tensor.matmul(mem_t[64:128, 512:576], ones_t[:, 0:64],
                                 gb_t[:, 64:128], start=False, stop=True)
                r_t = epi.tile([64, QP], F32, tag="r")
                nc.vector.reciprocal(out=r_t, in_=mem_t[64:128, 0:QP])
                nc.tensor.ldweights(mem_sb[:, 0:1])
                nc.tensor.matmul(mem_t[0:64, 0:512], kmq_t[:, w1c:w1c + OUT_CH],
                                 mem_sb[:, 0:512], start=True, stop=True)
                nc.tensor.matmul(mem_t[0:64, 512:576], kmq_t[:, w1c:w1c + OUT_CH],
                                 mem_sb[:, 512:QP], start=True, stop=True)
                u_t = epi.tile([64, QP], F32, tag="u")
                nc.vector.tensor_mul(u_t, mem_t[0:64, 0:QP], r_t)
                v_t = epi.tile([64, QP], F32, tag="v")
                nc.vector.tensor_add(v_t, u_t, y2_sb)
                nc.vector.tensor_scalar(
                    out=o_t[:, qs:qs + QP], in0=v_t, scalar1=shift_t,
                    scalar2=0.0, op0=mybir.AluOpType.add,
                    op1=mybir.AluOpType.max)

            LAGP = 2
            epi_state = None
            for rep in range(reps):
              for p in range(2):
                  qs = p * QP
                  mem_t = mpool.tile([128, 1024], F32, tag="mem")
                  g_t = daccp.tile([128, XSPLIT], BF16, tag="g")
                  gb_t = daccp.tile([128, QH - XSPLIT], BF16, tag="gb")
                  p_by_pr = {}
                  y2_sb = None

                  def emit_y2(qs=qs):
                      # y2 = W2 @ V_Q depends only on inputs: compute into a
                      # rotated score slot, stash to SBUF
                      y2t = spool.tile([128, 1024], F32, tag="s")
                      nc.tensor.matmul(y2t[0:64, 0:512], w2t_t[:, 0:64],
                                       vq_t[:, qs:qs + 512],
                                       start=True, stop=True)
                      nc.tensor.matmul(y2t[0:64, 512:576], w2t_t[:, 0:64],
                                       vq_t[:, qs + 512:qs + QP],
                                       start=True, stop=True)
                      y2_sb = epi.tile([64, QP], F32, tag="y2")
                      nc.vector.tensor_copy(out=y2_sb, in_=y2t[0:64, 0:QP])
                      return y2_sb

                  first_pass = (rep == 0 and p == 0)
                  if not first_pass:
                      y2_sb = emit_y2()
                  for pr in range(NPAIR + LAGP):
                    if first_pass and pr == 4:
                        # deferred past the ramp so y2's vqw wait doesn't
                        # stall the QK stream head
                        y2_sb = emit_y2()
                    if pr == 2 and epi_state is not None:
                        # previous pass's epilogue, deferred so its PE/DVE
                        # chain overlaps this pass's pipeline ramp
                        emit_epilogue(*epi_state)
                        epi_state = None
                    if pr < NPAIR:
                      # score slot split in two tiles (dep tracking is
                      # tile-granular; ACT and DVE readers must not share):
                      # sv_m 2 banks [e512|o512] for ACT, sv_t 1 bank
                      # [e64 o64] for the DVE schraudolph tail
                      sv_m = spool.tile([128, 1024], F32, tag="s")
                      # single-bank PSUM matmul targets fault on HW: tails
                      # live in one 2-bank tile, e in bank0, o in bank1
                      sv_t = spool.tile([128, 2, 512], F32, tag="st", bufs=1)
                      cl = pr * 128
                      lhs_e = kmq_t[0:64, cl:cl + 128]
                      lhs_o = kmq_t[64:128, cl:cl + 128]
                      nc.tensor.matmul(
                          sv_m[:, 0:512], lhs_e,
                          kmq_t[0:64, kq0 + qs:kq0 + qs + 512],
                          start=True, stop=True)
                      nc.tensor.matmul(
                          sv_m[:, 512:1024], lhs_o,
                          kmq_t[64:128, kq0 + qs:kq0 + qs + 512],
                          start=True, stop=True)
                      nc.tensor.matmul(
                          sv_t[:, 0:1, 0:64], lhs_e,
                          kmq_t[0:64, kq0 + qs + 512:kq0 + qs + QP],
                          start=True, stop=True)
                      nc.tensor.matmul(
                          sv_t[:, 1:2, 0:64], lhs_o,
                          kmq_t[64:128, kq0 + qs + 512:kq0 + qs + QP],
                          start=True, stop=True)
                      if p == 0 and pr == 0 and rep == 0:
                          # absorbers: PE observes the vt DMA and the DVE
                          # (vqw copy + ones memset) with one wait each
                          nc.tensor.ldweights(vt_t[:, 0:1])
                          nc.tensor.ldweights(ones_t[:, 0:1])
                      p_t = pp.tile([128, XSPLIT], BF16, tag="p")
                      nc.scalar.activation(out=p_t, in_=sv_m, func=AF.Exp)
                      pi_t = pq.tile([128, 2, 64], I16, tag="pi")
                      nc.vector.tensor_scalar(
                          out=pi_t, in0=sv_t[:, :, 0:64],
                          scalar1=SCHRAUD_A, scalar2=SCHRAUD_B,
                          op0=mybir.AluOpType.mult,
                          op1=mybir.AluOpType.add)
                      p_by_pr[pr] = (p_t, pi_t.bitcast(BF16).rearrange(
                          "p a b -> p (a b)"))
                    # denominator accumulation on DVE (bf16 2x), lagged one
                    # pair so a DVE schraudolph-exp isn't queued behind a
                    # fold that waits on ACT
                    fpr = pr - 1
                    if 0 <= fpr < NPAIR and pr <= NPAIR:
                        fp, fpi = p_by_pr[fpr]
                        if fpr == 0:
                            nc.vector.tensor_copy(out=g_t, in_=fp)
                            nc.vector.tensor_copy(out=gb_t, in_=fpi)
                        else:
                            nc.vector.tensor_add(g_t, g_t, fp)
                            nc.vector.tensor_add(gb_t, gb_t, fpi)
                    if pr >= LAGP:
                      pv = pr - LAGP
                      pvp, pvi = p_by_pr.pop(pv)
                      vl_e = vt_t[:, (2 * pv) * 128:(2 * pv + 1) * 128]
                      vl_o = vt_t[:, (2 * pv + 1) * 128:(2 * pv + 2) * 128]
                      st, sp = (pv == 0), (pv == NPAIR - 1)
                      nc.tensor.matmul(
                          mem_t[:, 0:512], vl_e, pvp[:, 0:512],
                          start=st, stop=False)
                      nc.tensor.matmul(
                          mem_t[:, 512:576], vl_e, pvi[:, 0:64],
                          start=st, stop=False)
                      nc.tensor.matmul(
                          mem_t[:, 0:512], vl_o, pvp[:, 512:1024],
                          start=False, stop=sp)
                      nc.tensor.matmul(
                          mem_t[:, 512:576], vl_o, pvi[:, 64:128],
                          start=False, stop=sp)

                  # only the mem->SBUF copy happens at pass end; the rest of
                  # the epilogue is deferred into the next pass's pipeline
                  mem_sb = epi.tile([128, QP], BF16, tag="mem_sb")
                  nc.vector.tensor_copy(out=mem_sb, in_=mem_t[:, 0:QP])
                  epi_state = (qs, mem_t, g_t, gb_t, mem_sb, y2_sb)
            emit_epilogue(*epi_state)
            nc.sync.dma_start(out=out, in_=o_t)


def _build_nc(reps=1):
    nc = bacc.Bacc("TRN2", target_bir_lowering=False, debug=False)
    kmq = nc.dram_tensor("kmq", [128, KMW + QH + OUT_CH], BF16,
                         kind="ExternalInput").ap()
    vt = nc.dram_tensor("vt", [128, M], BF16, kind="ExternalInput").ap()
    vqw = nc.dram_tensor("vqw", [128, QH + OUT_CH + 1], F32,
                         kind="ExternalInput").ap()
    out = nc.dram_tensor("out", [OUT_CH, QH], F32, kind="ExternalOutput").ap()
    _emit(nc, (kmq, vt, vqw, out), reps=reps)
    nc.compile()
    return nc


def prepare_in_maps(K_M, V_M, K_Q, V_Q, conv_w, bn_gamma, bn_beta, bn_mean, bn_var):
    """Host-side shard + layout prep. Returns list of 8 per-core input dicts."""
    bf16 = ml_dtypes.bfloat16
    K_M = np.asarray(K_M, np.float32)
    V_M = np.asarray(V_M, np.float32)
    K_Q = np.asarray(K_Q, np.float32)
    V_Q = np.asarray(V_Q, np.float32)
    conv_w = np.asarray(conv_w, np.float32)
    scale = np.asarray(bn_gamma, np.float32) / np.sqrt(
        np.asarray(bn_var, np.float32) + BN_EPS)
    shift = (np.asarray(bn_beta, np.float32)
             - np.asarray(bn_mean, np.float32) * scale)
    w_eff = conv_w * scale[:, None]
    w1t = np.ascontiguousarray(w_eff[:, :C_V].T)          # [128, 64]
    w2t = np.ascontiguousarray(w_eff[:, C_V:].T)          # [128, 64]

    in_maps = []
    for b in range(B):
        km_full = K_M[b].reshape(C_K, M)                  # [64, 9216]
        km_r = km_full.reshape(C_K, MT, 128)
        km_packed = np.empty((128, KMW), np.float32)
        km_packed[0:64] = km_r[:, 0::2, :].reshape(C_K, -1)
        km_packed[64:128] = km_r[:, 1::2, :].reshape(C_K, -1)

        v_full = V_M[b].reshape(C_V, M)
        vt = np.ascontiguousarray(
            v_full.reshape(C_V, MT, 128).transpose(2, 1, 0).reshape(128, M)
        ).astype(bf16)

        kq_full = K_Q[b].reshape(C_K, Q) * (1.0 / np.sqrt(C_K))
        vq_full = V_Q[b].reshape(C_V, Q)
        for h in range(2):
            sl = slice(h * QH, (h + 1) * QH)
            kq_half = kq_full[:, sl]
            kmq = np.empty((128, KMW + QH + OUT_CH), np.float32)
            kmq[:, 0:KMW] = km_packed
            kmq[0:64, KMW:KMW + QH] = kq_half
            kmq[64:128, KMW:KMW + QH] = kq_half
            kmq[:, KMW + QH:] = w1t
            vqw = np.zeros((128, QH + OUT_CH + 1), np.float32)
            vqw[:, 0:QH] = vq_full[:, sl]
            vqw[:, QH:QH + OUT_CH] = w2t
            vqw[0:OUT_CH, QH + OUT_CH] = shift
            in_maps.append({
                "kmq": kmq.astype(bf16),
                "vt": vt,
                "vqw": vqw,
            })
    return in_maps


def assemble_output(results):
    """results: list of 8 dicts with 'out' [64, 1152] -> [4, 64, 48, 48] f32."""
    out = np.empty((B, OUT_CH, Q), np.float32)
    for c in range(NCORES):
        b, h = c // 2, c % 2
        out[b, :, h * QH:(h + 1) * QH] = results[c]["out"]
    return out.reshape(B, OUT_CH, H, W)


_RUNNERS = {}


def _get_runner(reps=1):
    """Build the Bass module + a cached sharded jit callable (compile once)."""
    if reps in _RUNNERS:
        return _RUNNERS[reps]
    import jax
    from jax.sharding import Mesh, PartitionSpec
    from jax.experimental.shard_map import shard_map
    from concourse import bass2jax

    nc = _build_nc(reps=reps)
    bass2jax.install_neuronx_cc_hook()

    partition_name = nc.partition_id_tensor.name if nc.partition_id_tensor else None
    in_names, out_names, out_avals, zero_outs = [], [], [], []
    for alloc in nc.m.functions[0].allocations:
        if not isinstance(alloc, mybir.MemoryLocationSet):
            continue
        name = alloc.memorylocations[0].name
        if alloc.kind == "ExternalInput":
            if name != partition_name:
                in_names.append(name)
        elif alloc.kind == "ExternalOutput":
            out_names.append(name)
            shape = tuple(alloc.tensor_shape)
            dtype = mybir.dt.np(alloc.dtype)
            out_avals.append(jax.core.ShapedArray(shape, dtype))
            zero_outs.append(np.zeros(shape, dtype))
    n_params = len(in_names)
    n_outs = len(out_avals)
    all_in_names = in_names + out_names
    if partition_name is not None:
        all_in_names = all_in_names + [partition_name]

    def _body(*args):
        operands = list(args)
        if partition_name is not None:
            operands.append(bass2jax.partition_id_tensor())
        outs = bass2jax._bass_exec_p.bind(
            *operands,
            out_avals=tuple(out_avals),
            in_names=tuple(all_in_names),
            out_names=tuple(out_names),
            lowering_input_output_aliases=(),
            sim_require_finite=True,
            sim_require_nnan=True,
            nc=nc,
        )
        return tuple(outs)

    devices = jax.devices()[:NCORES]
    assert len(devices) == NCORES, f"need {NCORES} devices, got {len(jax.devices())}"
    mesh = Mesh(np.asarray(devices), ("core",))
    in_specs = (PartitionSpec("core"),) * (n_params + n_outs)
    out_specs = (PartitionSpec("core"),) * n_outs
    donate = tuple(range(n_params, n_params + n_outs))
    sharded = jax.jit(
        shard_map(_body, mesh=mesh, in_specs=in_specs, out_specs=out_specs,
                  check_rep=False),
        donate_argnums=donate, keep_unused=True,
    )
    _RUNNERS[reps] = (sharded, in_names, out_names, out_avals, zero_outs)
    return _RUNNERS[reps]


def run_cores(in_maps):
    """Run the 8-core SPMD program; returns per-core output dicts."""
    sharded, in_names, out_names, out_avals, zero_outs = _get_runner()
    concat_in = [
        np.concatenate([np.asarray(in_maps[c][n]) for c in range(NCORES)], axis=0)
        for n in in_names
    ]
    concat_zeros = [
        np.zeros((NCORES * z.shape[0], *z.shape[1:]), z.dtype) for z in zero_outs
    ]
    out_arrs = sharded(*concat_in, *concat_zeros)
    return [
        {
            name: np.asarray(out_arrs[i]).reshape(NCORES, *out_avals[i].shape)[c]
            for i, name in enumerate(out_names)
        }
        for c in range(NCORES)
    ]


def kernel(K_M, V_M, K_Q, V_Q, conv_w, bn_gamma, bn_beta, bn_mean, bn_var):
    in_maps = prepare_in_maps(K_M, V_M, K_Q, V_Q, conv_w,
                              bn_gamma, bn_beta, bn_mean, bn_var)
    results = run_cores(in_maps)
    return assemble_output(results)


# revision 30
# speedup vs baseline: 2.4007x; 1.3217x over previous
"""Trainium2 Bass kernel for nn_MemoryReader (sparse_attention).

Reference computation (per batch b):
  s[m,q]  = sum_c K_M[b,c,m] * K_Q[b,c,q] / sqrt(64)        m in [0,9216), q in [0,2304)
  attn    = softmax over m
  mem[c,q]= sum_m V_M[b,c,m] * attn[m,q]                    c in [0,128)
  E       = concat([mem, V_Q[b]], ch)                       [256, q]
  out     = relu(bn_scale * (conv_w @ E) + bn_shift)        [64, q]

Sharding: 8 cores = (B=4) x (Q halves of 1152). Fully data-parallel, no
collectives. Within a core, Q is processed in 2 passes of 576 columns.

m-tiles are processed in PAIRS (even tile on partitions 0:64 of the packed
K, odd on 64:128 -> the two QK matmuls run concurrently in PE row groups).
Each pair's scores live in one 3-bank PSUM slot laid out
  [e512 | o512 | e64 o64 pad]  (cols 0:512 / 512:1024 / 1024:1152)
so ONE activation instruction computes exp for the whole pair (N=1152),
amortizing the ACT access overhead. Two slots rotate (6 banks) + the PV
accumulator (2 banks) fill all 8 PSUM banks.

The softmax denominator accumulates on DVE (g += p per pair, bf16 2x mode)
and is reduced over partitions by a ones-matmul at pass end. To balance
engine load, a subset of pairs computes exp on DVE instead of ACT via a
Schraudolph-style bit trick: int16 = rint(s*log2e*128 + (16256+C)) viewed
as bf16 is 2^(s*log2e) with ~2-4% element error; softmax cancels the
common mode and the attention branch is ~1% of the output magnitude, so
the end-to-end error stays ~2e-4 (validated vs the reference).

Per-core inputs (host-prepared):
  kmq [128, 5824] bf16 : cols 0:4608   km_packed (even m-tiles on
                         partitions 0-63, odd on 64-127, contract=64)
                         cols 4608:5760 kq_dup (K_Q slice, pre-scaled by
                         1/sqrt(64), duplicated on partitions 64-127)
                         cols 5760:5824 w1t (BN-folded conv W for the mem
                         half, transposed)
  vt  [128, 9216] bf16 : V_M transposed per 128-tile: vt[p, t*128+c] =
                         V_M[c, t*128+p]  (PV lhsT, contract on m)
  vqw [128, 1217] f32  : cols 0:1152 V_Q slice, cols 1152:1216 w2t,
                         col 1216 (rows 0:64) BN shift
Output: out [64, 1152] f32.
"""

import numpy as np
import ml_dtypes

import concourse.bass as bass
from concourse import bacc
import concourse.mybir as mybir
import concourse.tile as tile
from concourse.tile_rust import add_dep_helper

B, C_K, C_V, NN, H, W = 4, 64, 128, 4, 48, 48
M = NN * H * W          # 9216
Q = H * W               # 2304
QH = Q // 2             # 1152 per core
QP = QH // 2            # 576 per in-kernel pass
OUT_CH = 64
BN_EPS = 1e-5
NCORES = 8
MT = M // 128           # 72 m-tiles
NPAIR = MT // 2         # 36 m-tile pairs per pass
KMW = (MT // 2) * 128   # 4608
BF16 = mybir.dt.bfloat16
F32 = mybir.dt.float32
I16 = mybir.dt.int16
AF = mybir.ActivationFunctionType

# Schraudolph exp: int16 bits = s*log2e*128 + 16256 + C viewed as bf16
SCHRAUD_A = float(np.float32(np.log2(np.e) * 128.0))
SCHRAUD_B = float(np.float32(16256.0 - 7.5))
# exp split point: ACT exps cols [0:XSPLIT] natively, DVE does the rest
# via the Schraudolph bit trick (engine load balance)
XSPLIT = 1024


def _emit(nc, aps, reps=1):
    kmq, vt, vqw, out = aps
    with tile.TileContext(nc) as tc:
        with (
            tc.tile_pool(name="consts", bufs=1) as consts,
            tc.tile_pool(name="pp", bufs=5) as pp,
            tc.tile_pool(name="pq", bufs=5) as pq,
            tc.tile_pool(name="epi", bufs=2) as epi,
            tc.tile_pool(name="obuf", bufs=1) as obuf,
            tc.tile_pool(name="dacc", bufs=2) as daccp,
            tc.tile_pool(name="spool", bufs=2, space="PSUM") as spool,
            tc.tile_pool(name="mpool", bufs=1, space="PSUM") as mpool,
        ):
            kmq_t = consts.tile([128, KMW + QH + OUT_CH], BF16)
            vt_t = consts.tile([128, M], BF16)
            vqw_t = consts.tile([128, QH + OUT_CH + 1], F32)
            ones_t = consts.tile([128, OUT_CH], BF16)

            # chunked loads: early m-tiles unblock before the tails arrive
            nc.sync.dma_start(out=kmq_t[:, KMW:KMW + QP], in_=kmq[:, KMW:KMW + QP])
            nc.sync.dma_start(out=kmq_t[:, 0:1152], in_=kmq[:, 0:1152])
            nc.sync.dma_start(out=kmq_t[:, KMW + QP:], in_=kmq[:, KMW + QP:])
            nc.sync.dma_start(out=vqw_t, in_=vqw)
            nc.sync.dma_start(out=vt_t[:, 0:2304], in_=vt[:, 0:2304])
            nc.sync.dma_start(out=kmq_t[:, 1152:KMW], in_=kmq[:, 1152:KMW])
            for i in range(1, 4):
                nc.sync.dma_start(out=vt_t[:, i * 2304:(i + 1) * 2304],
                                  in_=vt[:, i * 2304:(i + 1) * 2304])
            # DVE rewrites vqw and then produces ones: a single DVE
            # semaphore covers both; PE observes it via one dummy ldweights
            vqw_cp = nc.vector.tensor_copy(out=vqw_t, in_=vqw_t)
            ones_ms = nc.vector.memset(ones_t, 1.0)
            add_dep_helper(ones_ms.ins, vqw_cp.ins, sync=False,
                           reason="ones memset after vqw copy")

            kq0 = KMW                     # kq columns inside kmq_t
            w1c = KMW + QH                # w1t columns inside kmq_t
            vq_t = vqw_t[:, 0:QH]
            w2t_t = vqw_t[:, QH:QH + OUT_CH]
            shift_t = vqw_t[0:OUT_CH, QH + OUT_CH:QH + OUT_CH + 1]

            o_t = obuf.tile([OUT_CH, QH], F32, tag="o")

            def emit_epi_a(qs, mem_t, g_t, gb_t, mem_sb, y2_sb):
                # d (softmax denominator): partition-reduce via ones-matmuls
                # into the freed mem banks (partitions 64:128)
                for i in range(4):
                    nc.tensor.matmul(mem_t[64:128, 0:512], ones_t[:, 0:64],
                                     g_t[:, i * 512:(i + 1) * 512],
                                     start=(i == 0), stop=(i == 3))

            def emit_epi_b(qs, mem_t, g_t, gb_t, mem_sb, y2_sb):
                for i in range(4):
                    nc.tensor.matmul(mem_t[64:128, 512:576], ones_t[:, 0:64],
                                     gb_t[:, i * 64:(i + 1) * 64],
                                     start=(i == 0), stop=(i == 3))
                r_t = epi.tile([64, QP], F32, tag="r")
                nc.vector.reciprocal(out=r_t, in_=mem_t[64:128, 0:QP])
                return r_t

            def emit_epi_c(r_t, qs, mem_t, g_t, gb_t, mem_sb, y2_sb):
                nc.tensor.ldweights(mem_sb[:, 0:1])
                nc.tensor.matmul(mem_t[0:64, 0:512], kmq_t[:, w1c:w1c + OUT_CH],
                                 mem_sb[:, 0:512], start=True, stop=True)
                nc.tensor.matmul(mem_t[0:64, 512:576], kmq_t[:, w1c:w1c + OUT_CH],
                                 mem_sb[:, 512:QP], start=True, stop=True)
                u_t = epi.tile([64, QP], F32, tag="u")
                nc.vector.tensor_mul(u_t, mem_t[0:64, 0:QP], r_t)
                v_t = epi.tile([64, QP], F32, tag="v")
                nc.vector.tensor_add(v_t, u_t, y2_sb)
                nc.vector.tensor_scalar(
                    out=o_t[:, qs:qs + QP], in0=v_t, scalar1=shift_t,
                    scalar2=0.0, op0=mybir.AluOpType.add,
                    op1=mybir.AluOpType.max)

            LAGP = 2
            PV_START = 6
            NCPL = NPAIR // 2
            deferred = {}
            for rep in range(reps):
              for p in range(2):
                  qs = p * QP
                  mem_t = mpool.tile([128, 1024], F32, tag="mem")
                  g_t = daccp.tile([128, 2 * XSPLIT], BF16, tag="g")
                  gb_t = daccp.tile([128, 256], BF16, tag="gb")
                  p_by_pr = {}
                  couple_by_c = {}
                  state = {"pv_next": 0, "y2_sb": None, "qs": qs,
                           "mem_t": mem_t, "g_t": g_t, "gb_t": gb_t}

                  def emit_y2(qs=qs, state=state):
                      # y2 = W2 @ V_Q depends only on inputs: compute into a
                      # rotated score slot, stash to SBUF
                      y2t = spool.tile([128, 1024], F32, tag="s")
                      nc.tensor.matmul(y2t[0:64, 0:512], w2t_t[:, 0:64],
                                       vq_t[:, qs:qs + 512],
                                       start=True, stop=True)
                      nc.tensor.matmul(y2t[0:64, 512:576], w2t_t[:, 0:64],
                                       vq_t[:, qs + 512:qs + QP],
                                       start=True, stop=True)
                      y2_sb = epi.tile([64, QP], F32, tag="y2")
                      nc.vector.tensor_copy(out=y2_sb, in_=y2t[0:64, 0:QP])
                      state["y2_sb"] = y2_sb

                  def emit_pv(pv, mem_t=mem_t, p_by_pr=p_by_pr):
                      pvp, pvi = p_by_pr.pop(pv)
                      vl_e = vt_t[:, (2 * pv) * 128:(2 * pv + 1) * 128]
                      vl_o = vt_t[:, (2 * pv + 1) * 128:(2 * pv + 2) * 128]
                      st, sp = (pv == 0), (pv == NPAIR - 1)
                      nc.tensor.matmul(
                          mem_t[:, 0:512], vl_e, pvp[:, 0:512],
                          start=st, stop=False)
                      nc.tensor.matmul(
                          mem_t[:, 512:576], vl_e, pvi[:, 0:64],
                          start=st, stop=False)
                      nc.tensor.matmul(
                          mem_t[:, 0:512], vl_o, pvp[:, 512:1024],
                          start=False, stop=sp)
                      nc.tensor.matmul(
                          mem_t[:, 512:576], vl_o, pvi[:, 64:128],
                          start=False, stop=sp)

                  def emit_fold(fc, g_t=g_t, gb_t=gb_t, couple_by_c=couple_by_c):
                      # couple denominator fold: g on DVE (bf16 2x),
                      # tail-g on GPSIMD
                      cp, cpi = couple_by_c.pop(fc)
                      cpv = cp.rearrange("p a b -> p (a b)")
                      cpiv = cpi.bitcast(BF16).rearrange("p a b c -> p (a b c)")
                      if fc == 0:
                          nc.vector.tensor_copy(out=g_t, in_=cpv)
                          nc.gpsimd.tensor_copy(out=gb_t, in_=cpiv)
                      else:
                          nc.vector.tensor_add(g_t, g_t, cpv)
                          nc.gpsimd.tensor_add(gb_t, gb_t, cpiv)

                  def emit_mem_sb(state=state):
                      mem_sb = epi.tile([128, QP], BF16, tag="mem_sb")
                      nc.vector.tensor_copy(out=mem_sb,
                                            in_=state["mem_t"][:, 0:QP])
                      state["mem_sb"] = mem_sb

                  def epi_a(state=state):
                      emit_epi_a(state["qs"], state["mem_t"], state["g_t"],
                                 state["gb_t"], state["mem_sb"], state["y2_sb"])

                  def epi_b(state=state):
                      state["r"] = emit_epi_b(
                          state["qs"], state["mem_t"], state["g_t"],
                          state["gb_t"], state["mem_sb"], state["y2_sb"])

                  def epi_c(state=state):
                      emit_epi_c(state["r"], state["qs"], state["mem_t"],
                                 state["g_t"], state["gb_t"], state["mem_sb"],
                                 state["y2_sb"])

                  first_pass = (rep == 0 and p == 0)
                  if not first_pass:
                      emit_y2()
                  for pr in range(NPAIR):
                    if first_pass and pr == 4:
                        # deferred past the ramp so y2's vqw wait doesn't
                        # stall the QK stream head
                        emit_y2()
                    # previous pass's tail PVs + epilogue, deferred and
                    # spread so their PE/DVE chains overlap this pass's ramp
                    for fn in deferred.pop(pr, ()):
                        fn()
                    if True:
                      # score slot split in two tiles (dep tracking is
                      # tile-granular; ACT and DVE readers must not share)
                      sv_m = spool.tile([128, 1024], F32, tag="s")
                      # single-bank PSUM matmul targets fault on HW: tails
                      # live in one 2-bank tile, e in bank0, o in bank1
                      sv_t = spool.tile([128, 2, 512], F32, tag="st", bufs=1)
                      cl = pr * 128
                      lhs_e = kmq_t[0:64, cl:cl + 128]
                      lhs_o = kmq_t[64:128, cl:cl + 128]
                      nc.tensor.matmul(
                          sv_m[:, 0:512], lhs_e,
                          kmq_t[0:64, kq0 + qs:kq0 + qs + 512],
                          start=True, stop=True)
                      nc.tensor.matmul(
                          sv_m[:, 512:1024], lhs_o,
                          kmq_t[64:128, kq0 + qs:kq0 + qs + 512],
                          start=True, stop=True)
                      nc.tensor.matmul(
                          sv_t[:, 0:1, 0:64], lhs_e,
                          kmq_t[0:64, kq0 + qs + 512:kq0 + qs + QP],
                          start=True, stop=True)
                      nc.tensor.matmul(
                          sv_t[:, 1:2, 0:64], lhs_o,
                          kmq_t[64:128, kq0 + qs + 512:kq0 + qs + QP],
                          start=True, stop=True)
                      if first_pass and pr == 0:
                          # absorbers: PE observes the vt DMA and the DVE
                          # (vqw copy + ones memset) with one wait each
                          nc.tensor.ldweights(vt_t[:, 0:1])
                          nc.tensor.ldweights(ones_t[:, 0:1])
                      half = pr % 2
                      if half == 0:
                          p2_t = pp.tile([128, 2, XSPLIT], BF16, tag="p")
                          pi2_t = pq.tile([128, 2, 2, 64], I16, tag="pi")
                      nc.scalar.activation(out=p2_t[:, half:half + 1, :],
                                           in_=sv_m, func=AF.Exp)
                      nc.vector.tensor_scalar(
                          out=pi2_t[:, half:half + 1, :, :],
                          in0=sv_t[:, :, 0:64],
                          scalar1=SCHRAUD_A, scalar2=SCHRAUD_B,
                          op0=mybir.AluOpType.mult,
                          op1=mybir.AluOpType.add)
                      pv_view = pi2_t.bitcast(BF16).rearrange(
                          "p a b c -> p (a b c)")
                      p_by_pr[pr] = (
                          p2_t.rearrange("p a b -> p (a b)")[
                              :, half * XSPLIT:(half + 1) * XSPLIT],
                          pv_view[:, half * 128:(half + 1) * 128])
                      if half == 1:
                          couple_by_c[pr // 2] = (p2_t, pi2_t)
                    for fn in deferred.pop((pr, "post"), ()):
                        fn()
                    # couple folds, lagged one pair behind the exps
                    if pr >= 2 and pr % 2 == 0 and pr // 2 - 1 < NCPL - 1:
                        emit_fold(pr // 2 - 1)
                    # PV start is delayed at pass head so the deferred
                    # epilogue's mem-bank drain overlaps the QK/exp ramp;
                    # catch up 2 PVs per iteration afterwards
                    if pr >= PV_START:
                      budget = 2
                      while (state["pv_next"] <= pr - LAGP
                             and state["pv_next"] < NPAIR - 2 and budget):
                        budget -= 1
                        emit_pv(state["pv_next"])
                        state["pv_next"] += 1

                  # everything else is deferred into the next pass
                  pvn = state["pv_next"]
                  deferred = {
                      0: [lambda pv=pv, f=emit_pv: f(pv)
                          for pv in range(pvn, NPAIR - 2)],
                      1: [lambda f=emit_pv: f(NPAIR - 2)],
                      2: [lambda f=emit_pv: f(NPAIR - 1),
                          lambda f=emit_fold: f(NCPL - 1)],
                      (2, "post"): [emit_mem_sb],
                      3: [epi_a],
                      4: [epi_b],
                      5: [epi_c],
                  }
            for k in [0, 1, 2, (2, "post"), 3, 4, 5]:
                for fn in deferred.get(k, ()):
                    fn()
            nc.sync.dma_start(out=out, in_=o_t)


def _build_nc(reps=1):
    nc = bacc.Bacc("TRN2", target_bir_lowering=False, debug=False)
    kmq = nc.dram_tensor("kmq", [128, KMW + QH + OUT_CH], BF16,
                         kind="ExternalInput").ap()
    vt = nc.dram_tensor("vt", [128, M], BF16, kind="ExternalInput").ap()
    vqw = nc.dram_tensor("vqw", [128, QH + OUT_CH + 1], F32,
                         kind="ExternalInput").ap()
    out = nc.dram_tensor("out", [OUT_CH, QH], F32, kind="ExternalOutput").ap()
    _emit(nc, (kmq, vt, vqw, out), reps=reps)
    nc.compile()
    return nc


def prepare_in_maps(K_M, V_M, K_Q, V_Q, conv_w, bn_gamma, bn_beta, bn_mean, bn_var):
    """Host-side shard + layout prep. Returns list of 8 per-core input dicts."""
    bf16 = ml_dtypes.bfloat16
    K_M = np.asarray(K_M, np.float32)
    V_M = np.asarray(V_M, np.float32)
    K_Q = np.asarray(K_Q, np.float32)
    V_Q = np.asarray(V_Q, np.float32)
    conv_w = np.asarray(conv_w, np.float32)
    scale = np.asarray(bn_gamma, np.float32) / np.sqrt(
        np.asarray(bn_var, np.float32) + BN_EPS)
    shift = (np.asarray(bn_beta, np.float32)
             - np.asarray(bn_mean, np.float32) * scale)
    w_eff = conv_w * scale[:, None]
    w1t = np.ascontiguousarray(w_eff[:, :C_V].T)          # [128, 64]
    w2t = np.ascontiguousarray(w_eff[:, C_V:].T)          # [128, 64]

    in_maps = []
    for b in range(B):
        km_full = K_M[b].reshape(C_K, M)                  # [64, 9216]
        km_r = km_full.reshape(C_K, MT, 128)
        km_packed = np.empty((128, KMW), np.float32)
        km_packed[0:64] = km_r[:, 0::2, :].reshape(C_K, -1)
        km_packed[64:128] = km_r[:, 1::2, :].reshape(C_K, -1)

        v_full = V_M[b].reshape(C_V, M)
        vt = np.ascontiguousarray(
            v_full.reshape(C_V, MT, 128).transpose(2, 1, 0).reshape(128, M)
        ).astype(bf16)

        kq_full = K_Q[b].reshape(C_K, Q) * (1.0 / np.sqrt(C_K))
        vq_full = V_Q[b].reshape(C_V, Q)
        for h in range(2):
            sl = slice(h * QH, (h + 1) * QH)
            kq_half = kq_full[:, sl]
            kmq = np.empty((128, KMW + QH + OUT_CH), np.float32)
            kmq[:, 0:KMW] = km_packed
            kmq[0:64, KMW:KMW + QH] = kq_half
            kmq[64:128, KMW:KMW + QH] = kq_half
            kmq[:, KMW + QH:] = w1t
            vqw = np.zeros((128, QH + OUT_CH + 1), np.float32)
            vqw[:, 0:QH] = vq_full[:, sl]
            vqw[:, QH:QH + OUT_CH] = w2t
            vqw[0:OUT_CH, QH + OUT_CH] = shift
            in_maps.append({
                "kmq": kmq.astype(bf16),
                "vt": vt,
                "vqw": vqw,
            })
    return in_maps


def assemble_output(results):
    """results: list of 8 dicts with 'out' [64, 1152] -> [4, 64, 48, 48] f32."""
    out = np.empty((B, OUT_CH, Q), np.float32)
    for c in range(NCORES):
        b, h = c // 2, c % 2
        out[b, :, h * QH:(h + 1) * QH] = results[c]["out"]
    return out.reshape(B, OUT_CH, H, W)


_RUNNERS = {}


def _get_runner(reps=1):
    """Build the Bass module + a cached sharded jit callable (compile once)."""
    if reps in _RUNNERS:
        return _RUNNERS[reps]
    import jax
    from jax.sharding import Mesh, PartitionSpec
    from jax.experimental.shard_map import shard_map
    from concourse import bass2jax

    nc = _build_nc(reps=reps)
    bass2jax.install_neuronx_cc_hook()

    partition_name = nc.partition_id_tensor.name if nc.partition_id_tensor else None
    in_names, out_names, out_avals, zero_outs = [], [], [], []
    for alloc in nc.m.functions[0].allocations:
        if not isinstance(alloc, mybir.MemoryLocationSet):
            continue
        name = alloc.memorylocations[0].name
        if alloc.kind == "ExternalInput":
            if name != partition_name:
                in_names.append(name)
        elif alloc.kind == "ExternalOutput":
            out_names.append(name)
            shape = tuple(alloc.tensor_shape)
            dtype = mybir.dt.np(alloc.dtype)
            out_avals.append(jax.core.ShapedArray(shape, dtype))
            zero_outs.append(np.zeros(shape, dtype))
    n_params = len(in_names)
    n_outs = len(out_avals)
    all_in_names = in_names + out_names
    if partition_name is not None:
        all_in_names = all_in_names + [partition_name]

    def _body(*args):
        operands = list(args)
        if partition_name is not None:
            operands.append(bass2jax.partition_id_tensor())
        outs = bass2jax._bass_exec_p.bind(
            *operands,
            out_avals=tuple(out_avals),
            in_names=tuple(all_in_names),
            out_names=tuple(out_names),
            lowering_input_output_aliases=(),
            sim_require_finite=True,
            sim_require_nnan=True,
            nc=nc,
        )
        return tuple(outs)

    devices = jax.devices()[:NCORES]
    assert len(devices) == NCORES, f"need {NCORES} devices, got {len(jax.devices())}"
    mesh = Mesh(np.asarray(devices), ("core",))
    in_specs = (PartitionSpec("core"),) * (n_params + n_outs)
    out_specs = (PartitionSpec("core"),) * n_outs
    donate = tuple(range(n_params, n_params + n_outs))
    sharded = jax.jit(
        shard_map(_body, mesh=mesh, in_specs=in_specs, out_specs=out_specs,
                  check_rep=False),
        donate_argnums=donate, keep_unused=True,
    )
    _RUNNERS[reps] = (sharded, in_names, out_names, out_avals, zero_outs)
    return _RUNNERS[reps]


def run_cores(in_maps):
    """Run the 8-core SPMD program; returns per-core output dicts."""
    sharded, in_names, out_names, out_avals, zero_outs = _get_runner()
    concat_in = [
        np.concatenate([np.asarray(in_maps[c][n]) for c in range(NCORES)], axis=0)
        for n in in_names
    ]
    concat_zeros = [
        np.zeros((NCORES * z.shape[0], *z.shape[1:]), z.dtype) for z in zero_outs
    ]
    out_arrs = sharded(*concat_in, *concat_zeros)
    return [
        {
            name: np.asarray(out_arrs[i]).reshape(NCORES, *out_avals[i].shape)[c]
            for i, name in enumerate(out_names)
        }
        for c in range(NCORES)
    ]


def kernel(K_M, V_M, K_Q, V_Q, conv_w, bn_gamma, bn_beta, bn_mean, bn_var):
    in_maps = prepare_in_maps(K_M, V_M, K_Q, V_Q, conv_w,
                              bn_gamma, bn_beta, bn_mean, bn_var)
    results = run_cores(in_maps)
    return assemble_output(results)
